# revision 1
# baseline (speedup 1.0000x reference)
"""Distributed Trainium2 Bass kernel for multi-head attention.

Problem: x[2,2048,2048] @ qkv_w[2048,6144] -> rope(q,k) -> softmax(qk^T/sqrt(d)) @ v
         -> concat heads -> @ out_w[2048,2048].

Sharding (8 cores): core i handles batch b = i//4 and head group g = i%4
(heads 4g..4g+3).  All inputs are cast to bf16 on the host (the device
kernel computed in bf16 anyway), halving DMA traffic and removing all
on-device convert ops.  Each core:
  1. qT,kT = (Wqk_g^T x_b^T) with rope applied          [8 x [128, 2048]]
     (K half before Q half per token chunk; first-round DMAs are
     interleaved k-major across the HWDGE and SWDGE paths because the
     shared HWDGE descriptor generator (~630ns/DMA) limits issue rate)
  2. v     = x_b @ Wv_g  (natural layout)               [16 x [128, 512]]
  3. per head h, query chunk jq (512 queries):
       S^T tile = k_ik q^T (keys on partitions), P = exp(S^T/sqrt(d)),
       out^T += v_ik^T P  (PSUM-accumulated over 16 key tiles),
       denominator: P tiles accumulated into two bf16 partial sums on the
       DVE (two interleaved chains), then 2 cheap ones^T@p_sum matmuls
       (vs. the baseline's 256 full-cost ones^T@P matmuls, ~54us of PE).
  4. AllGather attnT shards (bf16) within the 4-core batch group, split
     into two half-head collectives (heads 0-1 after h1, heads 2-3 after
     h3) so the gathered tiles stream back during the next chunk instead
     of serializing at the chunk boundary.
  5. out[:, 512g:512(g+1)] = attnT_full^T @ out_w[...] in bf16.
     Out-projection matmuls for chunk j-1 are interleaved one-per-
     iteration inside the attention loop of chunk j (PE work per
     iteration 639ns vs exp 612ns on the scalar engine -- without the
     filler the PE starves 186ns per tile).  Accumulation visits the
     first-AG head rows before the second-AG rows.
Engine roles in the attention phase: scalar=exp only; DVE=denominator
accumulation, reciprocal, normalize, PSUM->SBUF output copies, output
DMA issue; Pool=partition_broadcast + collectives only (SWDGE desc-gen
is ~1us/DMA and anything queued ahead of partition_broadcast stalls the
DVE chain); sync=all other DMA traffic.
Host: slices/transposes/casts inputs per core, concatenates output columns.
"""

import numpy as np
import ml_dtypes

from concourse import bacc, mybir, tile
from concourse.bass_utils import run_bass_kernel_spmd

B, N, HID = 2, 2048, 2048
H, D = 16, 128
G = 4              # head groups (tensor parallel within a batch group)
HG = H // G        # heads per group
QK_COLS = HG * D   # 512
NT = N // 128      # 16 token tiles
KT = HID // 128    # 16 hidden tiles
TC = 512           # free-dim chunk
NTC = N // TC      # 4
OC = HID // G      # 512 output columns per core

F32 = mybir.dt.float32
BF16 = mybir.dt.bfloat16
SCALE = float(1.0 / np.sqrt(D))
SWAP_MASK = [p ^ 1 for p in range(32)]  # adjacent-pair swap, uniform per 32-lane group

# per-chunk split of heads across the two AllGathers.  The last chunk is
# asymmetric (3+1): its second collective moves a single head, so the
# serial finish chain after the very last exp (accum -> pd -> recip ->
# broadcast -> normalize -> DMA -> gather -> load) overlaps with ~10us of
# first-half out-projection matmuls instead of stalling the PE.
HALVES = [((0, 1, 2), (3,))] * 4


def half_of(jq, hh):
    return 0 if hh in HALVES[jq][0] else 1


def k3_order(jq):
    # out-projection accumulation order: rows delivered by the first
    # half-head AllGather before the second's
    return [k3 for k3 in range(16) if k3 % 4 in HALVES[jq][0]] + [
        k3 for k3 in range(16) if k3 % 4 in HALVES[jq][1]
    ]

_NC = None
LAST_RESULT = None


def _build(collective=True):
    nc = bacc.Bacc("TRN2", target_bir_lowering=False, debug=False, num_devices=8)

    xT = nc.dram_tensor("xT", [HID, N], BF16, kind="ExternalInput")
    wqk = nc.dram_tensor("wqk", [HID, 2 * QK_COLS], BF16, kind="ExternalInput")
    wv = nc.dram_tensor("wv", [HID, QK_COLS], BF16, kind="ExternalInput")
    wo = nc.dram_tensor("wo", [HID, OC], BF16, kind="ExternalInput")
    cosT = nc.dram_tensor("cosT", [D, N], BF16, kind="ExternalInput")
    sinT = nc.dram_tensor("sinT", [D, N], BF16, kind="ExternalInput")
    out = nc.dram_tensor("out", [N, OC], F32, kind="ExternalOutput")

    with tile.TileContext(nc) as tc:
        with (
            tc.tile_pool(name="dram", bufs=1, space="DRAM") as dram,
            tc.tile_pool(name="pqkv", bufs=1) as pqkv,
        ):
            qkT = [pqkv.tile([128, N], BF16, name=f"qkT{m}", tag=f"qkT{m}") for m in range(2 * HG)]
            v_sb = [pqkv.tile([128, QK_COLS], BF16, name=f"v{t}", tag=f"v{t}") for t in range(NT)]
            # persistent: the last token chunk's Q-projection is deferred into
            # attention chunk 0 as PE filler (chunk 0 otherwise stalls
            # 186ns/tile against the scalar engine's exp and has no outproj
            # work yet); its rope output is first read by chunk-3 attention.
            wqkQ_sb = [
                pqkv.tile([128, QK_COLS], BF16, name=f"wqkQ{k}", tag=f"wqkQ{k}")
                for k in range(KT)
            ]
            xt3 = [
                pqkv.tile([128, TC], BF16, name=f"xt3_{k}", tag=f"xt3_{k}")
                for k in range(KT)
            ]
            cos_sb = pqkv.tile([D, N], BF16, name="cos_sb", tag="cos")
            sin_sb = pqkv.tile([D, N], BF16, name="sin_sb", tag="sin")

            # ---- stages 1+2: q,k (transposed, roped) and v (natural) ----
            with (
                tc.tile_pool(name="s1w", bufs=1) as s1w,
                tc.tile_pool(name="s1x", bufs=1) as s1x,
                tc.tile_pool(name="s1t", bufs=3) as s1t,
                tc.tile_pool(name="s1c", bufs=1) as s1c,
                tc.tile_pool(name="psqk", bufs=8, space="PSUM") as psqk,
            ):
                # wqk columns 0:512 are Q, 512:1024 are K; separate tiles so
                # the K halves (needed first) can be fetched first
                wqkK_sb = [
                    s1w.tile([128, QK_COLS], BF16, name=f"wqkK{k}", tag=f"wqkK{k}")
                    for k in range(KT)
                ]
                wv_sb = [
                    s1w.tile([128, QK_COLS], BF16, name=f"wv{k}", tag=f"wv{k}")
                    for k in range(KT)
                ]

                def load_xt(tcn, first=False):
                    tsl = slice(tcn * TC, (tcn + 1) * TC)
                    if tcn == NTC - 1:
                        xt = xt3  # persistent: feeds the deferred Q half
                    else:
                        xt = [
                            s1x.tile([128, TC], BF16, name=f"xt{k}", tag=f"xt{k}", bufs=2)
                            for k in range(KT)
                        ]
                    for k in range(KT):
                        if first:
                            # k-major interleave of (xt[k], wqkK[k]) pairs so
                            # tile pair k lands before the k-th matmul.  The
                            # HWDGE descriptor generator (~630ns/DMA, shared
                            # by sync+scalar) takes wqkK odds on sync and xt
                            # odds on scalar; SWDGE (pool) takes even pairs.
                            if k == 0:
                                nc.sync.dma_start(
                                    wqkK_sb[k][:],
                                    wqk[k * 128 : (k + 1) * 128, QK_COLS:],
                                )
                                nc.scalar.dma_start(
                                    xt[k][:], xT[k * 128 : (k + 1) * 128, tsl]
                                )
                            elif k % 2 == 0:
                                nc.gpsimd.dma_start(
                                    xt[k][:], xT[k * 128 : (k + 1) * 128, tsl]
                                )
                                nc.gpsimd.dma_start(
                                    wqkK_sb[k][:],
                                    wqk[k * 128 : (k + 1) * 128, QK_COLS:],
                                )
                            else:
                                nc.scalar.dma_start(
                                    xt[k][:], xT[k * 128 : (k + 1) * 128, tsl]
                                )
                                nc.sync.dma_start(
                                    wqkK_sb[k][:],
                                    wqk[k * 128 : (k + 1) * 128, QK_COLS:],
                                )
                        else:
                            deng = nc.sync if k % 2 == 0 else nc.scalar
                            deng.dma_start(xt[k][:], xT[k * 128 : (k + 1) * 128, tsl])
                    return xt

                xt = load_xt(0, first=True)
                # rope tables (needed ~16us in), Q weight halves (~20us+),
                # then V weights (~37us+), split across both HWDGE queues
                nc.sync.dma_start(cos_sb[:], cosT[:])
                nc.sync.dma_start(sin_sb[:], sinT[:])
                for k in range(KT):
                    deng = nc.gpsimd if k % 2 == 0 else nc.sync
                    deng.dma_start(
                        wqkQ_sb[k][:], wqk[k * 128 : (k + 1) * 128, :QK_COLS]
                    )
                # wv odds on sync, not scalar: the scalar queue must reach
                # the K-half PSUM-drain copies by ~22us or the V section
                # stalls on PSUM banks (in-order queue head-of-line)
                for k in range(KT):
                    deng = nc.gpsimd if k % 2 == 0 else nc.sync
                    deng.dma_start(wv_sb[k][:], wv[k * 128 : (k + 1) * 128, :])

                for tcn in range(NTC):
                    tsl = slice(tcn * TC, (tcn + 1) * TC)
                    # prefetch next chunk's x now -- arrival-paced loads at
                    # the K-half start otherwise stall the PE ~0.4us/tile
                    xt_next = load_xt(tcn + 1) if tcn + 1 < NTC else None
                    for half in (1, 0):  # K half first (earlier attention readiness)
                        if half == 0 and tcn == NTC - 1:
                            continue  # last chunk's Q half deferred to stage 3
                        wh = wqkK_sb if half == 1 else wqkQ_sb
                        psums = [
                            psqk.tile([128, TC], F32, name="psqk", tag="psqk")
                            for _ in range(4)
                        ]
                        if tcn == 0 and half == 1:
                            # consume HWDGE-delivered odd tiles before the
                            # slower SWDGE evens (accumulation order is free)
                            korder = [0] + list(range(1, KT, 2)) + list(range(2, KT, 2))
                        else:
                            korder = list(range(KT))
                        for ki, k in enumerate(korder):
                            for mi in range(4):
                                nc.tensor.matmul(
                                    psums[mi][:],
                                    wh[k][:, mi * 128 : (mi + 1) * 128],
                                    xt[k][:],
                                    start=(ki == 0),
                                    stop=(ki == KT - 1),
                                )
                        for mi in range(4):
                            m = half * 4 + mi
                            qsb = s1t.tile([128, TC], BF16, tag="qsb")
                            nc.scalar.activation(qsb[:], psums[mi][:], mybir.ActivationFunctionType.Copy)
                            shuf = s1t.tile([128, TC], BF16, tag="shuf")
                            nc.vector.stream_shuffle(shuf[:], qsb[:], SWAP_MASK)
                            t1 = s1t.tile([128, TC], F32, tag="t1")
                            nc.vector.tensor_tensor(
                                t1[:], qsb[:], cos_sb[:, tsl], mybir.AluOpType.mult
                            )
                            t2 = s1t.tile([128, TC], F32, tag="t2")
                            nc.vector.tensor_tensor(
                                t2[:], shuf[:], sin_sb[:, tsl], mybir.AluOpType.mult
                            )
                            nc.vector.tensor_tensor(
                                qkT[m][:, tsl], t1[:], t2[:], mybir.AluOpType.add
                            )
                    for mtl in range(4):
                        mt = tcn * 4 + mtl
                        pv = psqk.tile([128, QK_COLS], F32, name="psv", tag="psqk")
                        for k in range(KT):
                            nc.tensor.matmul(
                                pv[:],
                                xt[k][:, mtl * 128 : (mtl + 1) * 128],
                                wv_sb[k][:],
                                start=(k == 0),
                                stop=(k == KT - 1),
                            )
                        nc.vector.tensor_copy(v_sb[mt][:], pv[:])
                    xt = xt_next

            # ---- stages 3-5: attention; AG + output projection of chunk
            # j-1 interleaved into the attention loops of chunk j ----
            CHUNKS = [(0, 512), (512, 512), (1024, 512), (1536, 512)]
            NCH = len(CHUNKS)
            # two half-head collectives per chunk (split per HALVES)
            cc_in = [
                [dram.tile([len(HALVES[j][half]) * 128, w], BF16,
                           name=f"cc_in{j}{half}", tag=f"cc_in{j}{half}")
                 for half in range(2)]
                for j, (q0, w) in enumerate(CHUNKS)
            ]
            cc_out = [
                [dram.tile([G * len(HALVES[j][half]) * 128, w], BF16,
                           name=f"cc_out{j}{half}", tag=f"cc_out{j}{half}")
                 for half in range(2)]
                for j, (q0, w) in enumerate(CHUNKS)
            ]
            with (
                tc.tile_pool(name="s3p", bufs=4) as s3p,
                tc.tile_pool(name="s3r", bufs=2) as s3r,
                tc.tile_pool(name="s3c", bufs=1) as s3c,
                tc.tile_pool(name="s3a", bufs=2) as s3a,
                tc.tile_pool(name="s3s", bufs=2) as s3s,
                tc.tile_pool(name="s3d", bufs=4) as s3d,
                tc.tile_pool(name="s5w", bufs=1) as s5w,
                tc.tile_pool(name="s5at", bufs=2) as s5at,
                tc.tile_pool(name="s5o", bufs=5) as s5o,
                tc.tile_pool(name="pss", bufs=3, space="PSUM") as pss,
                tc.tile_pool(name="psd", bufs=1, space="PSUM") as psd,
                tc.tile_pool(name="pso", bufs=2, space="PSUM") as pso,
                tc.tile_pool(name="psf", bufs=2, space="PSUM") as psf,
            ):
                ones_f32 = s3c.tile([128, 1], F32, tag="ones_f32")
                nc.vector.memset(ones_f32[:], 1.0)
                ones_sb = s3c.tile([128, 1], BF16, tag="ones")
                nc.vector.tensor_copy(ones_sb[:], ones_f32[:])
                wo_sb = [s5w.tile([128, OC], BF16, name=f"wo{k}", tag=f"wo{k}") for k in range(KT)]
                for k in range(KT):
                    nc.sync.dma_start(wo_sb[k][:], wo[k * 128 : (k + 1) * 128, :])

                def attention_head(jq, h, filler=None):
                    """filler: list of thunks (PE matmul emissions) doled out
                    between the scores and AV matmuls so the PE has work
                    while the scalar engine computes exp."""
                    q0, w = CHUNKS[jq]
                    qsl = slice(q0, q0 + w)
                    po = pso.tile([128, w], F32, name="pso", tag="pso")
                    psum = s3s.tile([128, w], BF16, name="psum", tag="psum")
                    filler = list(filler) if filler else []
                    for ik in range(NT):
                        ps = pss.tile([128, w], F32, name="pss", tag="pss")
                        nc.tensor.matmul(
                            ps[:],
                            qkT[HG + h][:, ik * 128 : (ik + 1) * 128],
                            qkT[h][:, qsl],
                            start=True,
                            stop=True,
                        )
                        p = s3p.tile([128, w], BF16, name="p", tag="p")
                        nc.scalar.activation(
                            p[:], ps[:], mybir.ActivationFunctionType.Exp, scale=SCALE
                        )
                        if filler and ik != 12 and not (h == 0 and ik > 12):
                            # one filler per iteration, but skip iter 12 so
                            # the 4 second-AllGather rows (positions 12-15)
                            # get an extra ~0.6us of DMA-arrival slack; the
                            # leftover runs after the loop.  At h0 the B
                            # rows are still in flight from the previous
                            # chunk's last-head gather chain, so all 4 run
                            # as leftovers after the loop.
                            filler.pop(0)()
                        nc.tensor.matmul(
                            po[:],
                            v_sb[ik][:, h * 128 : (h + 1) * 128],
                            p[:],
                            start=(ik == 0),
                            stop=(ik == NT - 1),
                        )
                        if ik == 0:
                            nc.vector.tensor_copy(psum[:], p[:])
                        else:
                            nc.vector.tensor_tensor(
                                psum[:], psum[:], p[:], mybir.AluOpType.add
                            )
                    for f in filler:  # leftover from the skipped iteration
                        f()
                    return po, psum

                def attention_head_finish(jq, h, po, psum):
                    q0, w = CHUNKS[jq]
                    half = half_of(jq, h)
                    hh = HALVES[jq][half].index(h)
                    nheads = len(HALVES[jq][half])
                    pd = psd.tile([1, w], F32, name="psd", tag="psd")
                    nc.tensor.matmul(pd[:], ones_sb[:], psum[:], start=True, stop=True)
                    dr = s3d.tile([1, w], F32, name="dr", tag="dr")
                    nc.vector.reciprocal(dr[:], pd[:])
                    drb = s3d.tile([128, w], F32, name="drb", tag="drb")
                    nc.gpsimd.partition_broadcast(drb[:], dr[:])
                    asb = s3a.tile([128, w], BF16, name=f"asb{h}", tag=f"asb{h}")
                    nc.vector.tensor_tensor(asb[:], po[:], drb[:], mybir.AluOpType.mult)
                    nc.sync.dma_start(
                        cc_in[jq][half][hh * 128 : (hh + 1) * 128, :], asb[:]
                    )
                    if not collective:
                        # stand-in for this head's share of the half-head
                        # AllGather: fan the shard out to all 4 group rows
                        # now (split across SWDGE and HWDGE) instead of 4
                        # serial ~1us Pool desc-gens after the last head
                        for gg in range(G):
                            deng = nc.gpsimd if gg % 2 == 0 else nc.sync
                            deng.dma_start(
                                cc_out[jq][half][
                                    (gg * nheads + hh) * 128 : (gg * nheads + hh + 1) * 128, :
                                ],
                                cc_in[jq][half][hh * 128 : (hh + 1) * 128, :],
                            )

                def allgather(jq, half):
                    # stand-in traffic is emitted per-head in
                    # attention_head_finish when collective=False
                    if collective:
                        nc.gpsimd.collective_compute(
                            "AllGather",
                            mybir.AluOpType.bypass,
                            replica_groups=[[0, 1, 2, 3], [4, 5, 6, 7]],
                            ins=[cc_in[jq][half].opt()],
                            outs=[cc_out[jq][half].opt()],
                        )

                atb_live = {}  # (jq, k3) -> SBUF lhsT tile for outproj
                pending_outs = []  # (ob tile, mq): DMA issue deferred until
                # the ob is ready, so a waiting out-write can't head-of-line
                # block asb/atb DMAs on the sync queue

                def flush_outs():
                    for ob, mq in pending_outs:
                        nc.sync.dma_start(out[mq * 128 : (mq + 1) * 128, :], ob[:])
                    pending_outs.clear()

                def load_atb(jq, half, queues=(None,)):
                    q0, w = CHUNKS[jq]
                    nheads = len(HALVES[jq][half])
                    i = 0
                    for k3 in range(KT):
                        g, hh = k3 // HG, k3 % HG
                        if half_of(jq, hh) != half:
                            continue
                        t = s5at.tile([128, w], BF16, name=f"at{k3}", tag=f"at{k3}")
                        r = g * nheads + HALVES[jq][half].index(hh)
                        deng = queues[i % len(queues)] or nc.sync
                        deng.dma_start(
                            t[:], cc_out[jq][half][r * 128 : (r + 1) * 128, :]
                        )
                        atb_live[(jq, k3)] = t
                        i += 1

                def outproj_thunks(jq, mql, positions=range(KT), pf=None, defer=True):
                    """PE-matmul thunks (one per k3 position, first-AG rows
                    first) accumulating one 128-row output tile; the last
                    position drains PSUM via the DVE."""
                    q0, w = CHUNKS[jq]
                    mq = q0 // 128 + mql
                    if pf is None:
                        pf = psf.tile([128, OC], F32, name="psf", tag="psf")

                    order = k3_order(jq)

                    def mk(pos):
                        k3 = order[pos]

                        def thunk():
                            nc.tensor.matmul(
                                pf[:],
                                atb_live[(jq, k3)][:, mql * 128 : (mql + 1) * 128],
                                wo_sb[k3][:],
                                start=(pos == 0),
                                stop=(pos == KT - 1),
                            )
                            if pos == KT - 1:
                                ob = s5o.tile([128, OC], F32, name="ob", tag="ob")
                                nc.vector.tensor_copy(ob[:], pf[:])
                                if defer:
                                    pending_outs.append((ob, mq))
                                else:
                                    # tail: alternate queues so one write's
                                    # wait can't serialize the rest (scalar
                                    # is exp-free by now)
                                    deng = nc.sync if mql % 2 == 0 else nc.scalar
                                    deng.dma_start(
                                        out[mq * 128 : (mq + 1) * 128, :], ob[:]
                                    )
                        return thunk

                    return [mk(pos) for pos in positions], pf

                def deferred_q_thunks(mi):
                    """The last token chunk's Q projection for head-group
                    column mi: 16 matmuls into a borrowed psf bank (outproj
                    doesn't run during chunk 0), then the PSUM->bf16 copy and
                    rope, writing qkT[mi][:, 1536:2048] (first read by
                    chunk-3 attention)."""
                    tsl = slice((NTC - 1) * TC, NTC * TC)
                    pq = psf.tile([128, TC], F32, name="psq", tag="psf")

                    def mk(k):
                        def thunk():
                            nc.tensor.matmul(
                                pq[:],
                                wqkQ_sb[k][:, mi * 128 : (mi + 1) * 128],
                                xt3[k][:],
                                start=(k == 0),
                                stop=(k == KT - 1),
                            )
                            if k == KT - 1:
                                qsb = s3r.tile([128, TC], BF16, tag="qsb")
                                nc.scalar.activation(
                                    qsb[:], pq[:], mybir.ActivationFunctionType.Copy
                                )
                                shuf = s3r.tile([128, TC], BF16, tag="shuf")
                                nc.vector.stream_shuffle(shuf[:], qsb[:], SWAP_MASK)
                                t1 = s3r.tile([128, TC], F32, tag="t1")
                                nc.vector.tensor_tensor(
                                    t1[:], qsb[:], cos_sb[:, tsl], mybir.AluOpType.mult
                                )
                                t2 = s3r.tile([128, TC], F32, tag="t2")
                                nc.vector.tensor_tensor(
                                    t2[:], shuf[:], sin_sb[:, tsl], mybir.AluOpType.mult
                                )
                                nc.vector.tensor_tensor(
                                    qkT[mi][:, tsl], t1[:], t2[:], mybir.AluOpType.add
                                )
                        return thunk

                    return [mk(k) for k in range(KT)]

                for jq in range(NCH):
                    if jq >= 1:
                        load_atb(jq - 1, 1, queues=(nc.sync, nc.gpsimd))
                        flush_outs()
                    for h in range(HG):
                        if jq == 0:
                            # cross-assigned (head h builds Q for head h+1)
                            # so the filler never writes the qkT tile the
                            # current head's scores are reading
                            filler = deferred_q_thunks((h + 1) % HG)
                        else:
                            filler = outproj_thunks(jq - 1, h)[0]
                        po, psum_ab = attention_head(jq, h, filler)
                        attention_head_finish(jq, h, po, psum_ab)
                        if h == HALVES[jq][0][-1]:
                            allgather(jq, 0)
                            load_atb(jq, 0)
                    allgather(jq, 1)
                # tail: last chunk's projection.  Run all four first-half
                # accumulations (12 of 16 rows each -- the last chunk's
                # second AllGather moves a single head) while that gather +
                # loads fly, borrowing pss-pool banks, then the remainders.
                load_atb(NCH - 1, 1, queues=(nc.sync, nc.scalar, nc.gpsimd, nc.sync))
                flush_outs()
                NA = G * len(HALVES[NCH - 1][0])
                tails = []
                for mql in range(4):
                    pfx = (
                        None if mql < 2
                        else pss.tile([128, OC], F32, name="pss", tag="pss")
                    )
                    thunks, pf = outproj_thunks(
                        NCH - 1, mql, positions=range(NA), pf=pfx, defer=False
                    )
                    for f in thunks:
                        f()
                    tails.append(pf)
                for mql in range(4):
                    thunks, _ = outproj_thunks(
                        NCH - 1, mql, positions=range(NA, KT),
                        pf=tails[mql], defer=False,
                    )
                    for f in thunks:
                        f()

    nc.compile()
    return nc


def _get_nc():
    global _NC
    if _NC is None:
        _NC = _build()
    return _NC


def _prep_in_maps(x, rope, qkv_w, out_w):
    x = np.asarray(x, np.float32)
    rope = np.asarray(rope, np.float32)
    qkv_w = np.asarray(qkv_w, np.float32)
    out_w = np.asarray(out_w, np.float32)

    bf = ml_dtypes.bfloat16
    freqs = rope[:, 0, :]  # [N, D]
    cosT = np.ascontiguousarray(np.repeat(freqs[:, 0::2], 2, axis=1).T).astype(bf)
    sinT = np.repeat(freqs[:, 1::2], 2, axis=1).T.copy()
    sinT[0::2, :] *= -1.0  # rope sign folded in: rot[2i] = -q[2i+1]
    sinT = np.ascontiguousarray(sinT).astype(bf)

    qkv3 = qkv_w.reshape(HID, 3, H, D)
    xTs = [np.ascontiguousarray(x[b].T).astype(bf) for b in range(B)]
    in_maps = []
    for core in range(8):
        b, g = core // G, core % G
        hs = slice(g * HG, (g + 1) * HG)
        wq = qkv3[:, 0, hs, :].reshape(HID, QK_COLS)
        wk = qkv3[:, 1, hs, :].reshape(HID, QK_COLS)
        in_maps.append(
            dict(
                xT=xTs[b],
                wqk=np.ascontiguousarray(np.concatenate([wq, wk], axis=1)).astype(bf),
                wv=np.ascontiguousarray(qkv3[:, 2, hs, :].reshape(HID, QK_COLS)).astype(bf),
                wo=np.ascontiguousarray(out_w[:, g * OC : (g + 1) * OC]).astype(bf),
                cosT=cosT,
                sinT=sinT,
            )
        )
    return in_maps


def kernel(x, rope, qkv_w, out_w):
    global LAST_RESULT
    nc = _get_nc()
    in_maps = _prep_in_maps(x, rope, qkv_w, out_w)
    res = run_bass_kernel_spmd(nc, in_maps, core_ids=list(range(8)))
    LAST_RESULT = res
    outs = [r["out"] for r in res.results]
    full = np.stack(
        [np.concatenate([outs[b * G + g] for g in range(G)], axis=1) for b in range(B)]
    )
    return full.astype(np.float32)

